# revision 17
# baseline (speedup 1.0000x reference)
"""Trainium2 Bass kernel for nn_NetworkODEModel (gnn_message_passing).

Reference computation (B=64, N=128, D=2, H=64):
  node_out = MLP_node(x)                                  # [B,N,1]
  c[b,i,j] = MLP_coup(cat(x[b,i], x[b,j]))                # [B,N,N,1]
  A        = sigmoid(A_param - I/eps)
  coup[b,i] = sum_j A[i,j] * c[b,i,j]
  out[...,0] = x[...,1];  out[...,1] = node_out + coup

Data-parallel over batch (8 cores x 8 batches); all O(B*N^2*H) work stays
in SBUF in bf16.  Per-quad tile = [128 part, 512 cols]: partitions carry two
i-streams (rows 0:64 = features of i=2p, 64:128 = i=2p+1), columns carry 4
pairs x 128 j.

Engine balance (the whole point of this structure).  Two interchangeable
layer-1 paths, split N_GQ / (16 - N_GQ) quads per batch to balance PE vs DVE:
  * GQ path: LeakyReLU(z) = 0.99*relu(z) + 0.01*z.  t1 = relu(v_j + u_i) =
    ONE dual-op tensor_scalar (op0=add, op1=max 0) per pair -- 4x DVE mode in
    bf16.  The 0.01*z linear part is rank-6 in (x_i, x_j) and rides a tiny
    accumulating matmul (GQ, 6-row stationary, moving tile MQ) into the same
    PSUM bank as layer 2 (stationary 0.99*blockdiag(W2,W2), ACT bias cc).
  * DVE path: exact lrelu on DVE -- 4x tensor_scalar z1-builds plus ONE
    scalar_tensor_tensor max(0.01*z1, z1) over the 512-wide quad, then a
    single unscaled blockdiag(W2,W2) matmul (ACT bias bc2).  One less PE
    matmul + LDWEIGHTS per quad at the cost of ~330ns more DVE.
  * Layer-2 LeakyReLU = ONE ScalarE Prelu(alpha=0.01, bias per path) per
    2-quad [128,1024] PSUM supertile, straight to SBUF bf16 -- ACT is the
    only cheap PSUM evictor (DVE fp32-PSUM ops run at 1x), and grouping
    amortizes its ~440-cycle access-latency init.
  * Layer 3 (Wco contraction + scatter to S[i,j]) = ONE [128,512] matmul per
    quad against a sliding 8-wide Wco strip, PSUM-accumulated over quads.
    Off-block columns produce garbage that the epilogue masks for free:
    coup = sum((S * Mmask), j) with Mmask[i,128k+j] = A[i,j]*(k == k(i)).
  * Per-batch epilogue on DVE: S*Mmask multiply, free-axis reduce, node add.
PE's L3 is software-pipelined (lags L3LAG quads) so PE rarely waits on ACT.
Host precomputes every linear map (u, v, GQ/W2a/W2b/strip/Mmask/node).
walrus encodes at most ONE sync wait per instruction -> _split_multiwaits
hoists extras onto same-engine NoOps.
"""

import sys

for _p in ("/opt/trn_rl_repo",):
    if _p not in sys.path:
        sys.path.insert(0, _p)

import numpy as np

import concourse.bass as bass
import concourse.mybir as mybir
import concourse.tile as tile
from concourse.bass_utils import run_bass_kernel_spmd

F32 = mybir.dt.float32
BF16 = mybir.dt.bfloat16
ALU = mybir.AluOpType
ACTF = mybir.ActivationFunctionType

NCORES = 8
B, N, D, H = 64, 128, 2, 64
BL = B // NCORES            # batches per core = 8
NPAIR = N // 2              # i-pairs per batch = 64
QUAD = 4                    # i-pairs per tile
NQ = NPAIR // QUAD          # 16 quads per batch
EPS = 1e-5
SLOPE = 0.01                # torch LeakyReLU default
L3LAG = 3                   # quads of software pipelining for the L3 matmul
GRP = 2                     # quads per ACT activation (PSUM supertile)
N_GQ = 8                    # quads per batch on the GQ-matmul path (rest DVE;
                            # must be a multiple of GRP so ACT groups stay
                            # bias-homogeneous).  8/8 balances PE vs DVE:
                            # measured 73us vs 138us (16/0) and 190us (0/16)

BN = BL * N                 # 1024 (b,j) columns per core
STRIPW = 8 * (NQ - 1) + 128  # 248: sliding 8-wide Wco window

# ---- f32 constants layout [128, CF_W] ----
OFF_U2 = 0                  # [128, 512]  u vectors, col = 64*b + p
OFF_MM = 512                # [128, 512]  A-mask for the S epilogue
OFF_NODE = 1024             # [128, 8]    node_out + bco*rowsum(A), [i, b]
OFF_XB1 = 1032              # [128, 8]    x[b, n, 1] as [n, b]
OFF_CC = 1040               # [128, 1]    layer-2 bias, GQ path (ACT bias port)
OFF_B2 = 1041               # [128, 1]    layer-2 bias, DVE-lrelu path
CF_W = 1042

# ---- bf16 constants layout [128, CB_W] ----
OFF_VV = 0                  # [128, 1024] [v_j; v_j], col = 128*b + j
OFF_W2A = 1024              # [128, 128]  0.99 * blockdiag(W2, W2)   (GQ path)
OFF_GQ = 1152               # [6, 128]    0.01 * [Gb; Ga|0; 0|Ga]
OFF_STRIP = 1280            # [128, 248]  sliding Wco strip
OFF_W2B = 1528              # [128, 128]  blockdiag(W2, W2)          (DVE path)
CB_W = 1656

MQ_W = BL * NQ * 512        # 65536 moving columns for the GQ matmul


def build_program(debug=False, split_waits=True, repeat=1):
    nc = bass.Bass("TRN2", target_bir_lowering=False, debug=debug)
    cf = nc.dram_tensor("cf", [128, CF_W], F32, kind="ExternalInput")
    cb = nc.dram_tensor("cb", [128, CB_W], BF16, kind="ExternalInput")
    mq = nc.dram_tensor("mq", [6, MQ_W], BF16, kind="ExternalInput")
    out = nc.dram_tensor("out", [BL, N, 2], F32, kind="ExternalOutput")

    with tile.TileContext(nc) as tc:
        _body(nc, tc, cf, cb, mq, out, repeat=repeat)
    if split_waits:
        _split_multiwaits(nc)
    nc.finalize()
    return nc


def _split_multiwaits(nc):
    """walrus on this stack encodes at most ONE sync wait per instruction;
    hoist extras onto same-engine NoOps."""
    import bass_rust
    n = 0
    for fn in nc.m.functions:
        for bb in fn.blocks:
            insts = bb.instructions
            changed = False
            out_list = []
            for inst in insts:
                si = inst.sync_info
                if si is not None and len(si.on_wait) > 1:
                    waits = list(si.on_wait)
                    for w in waits[:-1]:
                        nop = bass_rust.InstNoOp(name=f"ant-wait-split-{n}")
                        n += 1
                        nop.engine = inst.engine
                        nop.sync_info = bass_rust.SyncInfo(on_wait=[w], on_update=[])
                        out_list.append(nop)
                    inst.sync_info = bass_rust.SyncInfo(
                        on_wait=[waits[-1]], on_update=list(si.on_update))
                    changed = True
                out_list.append(inst)
            if changed:
                bb.instructions = out_list


def _body(nc, tc, cf, cb, mq, out, repeat=1):
    with (
        tc.tile_pool(name="const", bufs=1) as cpool,
        tc.tile_pool(name="t1p", bufs=6) as t1pool,
        tc.tile_pool(name="c2p", bufs=3) as c2pool,
        tc.tile_pool(name="zp", bufs=2) as zpool,
        tc.tile_pool(name="psum_c", bufs=3, space="PSUM") as ppool,
        tc.tile_pool(name="psum_s", bufs=2, space="PSUM") as spool,
    ):
        CF = cpool.tile([128, CF_W], F32, tag="cf")
        CB = cpool.tile([128, CB_W], BF16, tag="cb")
        MQ = cpool.tile([6, MQ_W], BF16, tag="mq")
        nc.sync.dma_start(CF[:, :], cf[:, :])
        nc.sync.dma_start(CB[:, :], cb[:, :])
        nc.sync.dma_start(MQ[:, :], mq[:, :])
        # absorb each DMA wait on DVE once so later DVE readers never pair a
        # DMA wait with a second wait
        dscr = cpool.tile([128, 2], F32, tag="dscr")
        nc.vector.tensor_copy(dscr[:, 0:1], CF[:, 0:1])
        nc.vector.tensor_copy(dscr[:, 1:2], CB[:, 0:1])
        nc.vector.tensor_copy(dscr[0:6, 0:1], MQ[:, 0:1])

        u2 = CF[:, OFF_U2:OFF_U2 + BL * NPAIR]
        Mmask = CF[:, OFF_MM:OFF_MM + 512]
        nodec = CF[:, OFF_NODE:OFF_NODE + BL]
        xb1 = CF[:, OFF_XB1:OFF_XB1 + BL]
        ccv = CF[:, OFF_CC:OFF_CC + 1]
        b2v = CF[:, OFF_B2:OFF_B2 + 1]
        vv = CB[:, OFF_VV:OFF_VV + BN]
        W2a = CB[:, OFF_W2A:OFF_W2A + 128]
        GQ = CB[0:6, OFF_GQ:OFF_GQ + 128]
        strip = CB[:, OFF_STRIP:OFF_STRIP + STRIPW]
        W2b = CB[:, OFF_W2B:OFF_W2B + 128]

        val_cols = cpool.tile([N, BL], F32, tag="val_cols")

        for _rep in range(repeat):
            pending = []   # (S_tile, q, c2l, b) awaiting L3 emission

            def emit_l3(job):
                S, q, idx, c2l, b = job
                nc.tensor.matmul(
                    S[:, :], strip[:, 8 * (NQ - 1 - q):8 * (NQ - 1 - q) + 128],
                    c2l[:, :], start=(idx == 0), stop=(idx == NQ - 1))
                if idx == NQ - 1:
                    # epilogue: coup = sum_j A*S (+ node column)
                    Z = zpool.tile([128, 512], F32, tag="Z")
                    nc.vector.tensor_tensor(Z[:, :], S[:, :], Mmask, op=ALU.mult)
                    rs = zpool.tile([128, 1], F32, tag="rs")
                    nc.vector.tensor_reduce(rs[:, :], Z[:, :],
                                            axis=mybir.AxisListType.X, op=ALU.add)
                    nc.vector.tensor_scalar(val_cols[:, b:b + 1], rs[:, :],
                                            nodec[:, b:b + 1], None, op0=ALU.add)

            # first N_GQ quads take the GQ-matmul path, the rest the DVE
            # path; each 2-quad ACT group stays path-homogeneous (the two
            # paths need different layer-2 bias vectors)
            order = list(range(NQ))
            for b in range(BL):
                S = spool.tile([128, 512], F32, tag="S")
                vb = vv[:, b * N:(b + 1) * N]
                for g in range(NQ // GRP):
                    # 2-quad supertile: matmuls fill both 512-col halves of a
                    # 2-bank PSUM tile; ONE ACT Prelu drains all 1024 cols
                    Cps = ppool.tile([128, GRP * 512], F32, tag="Cps")
                    c2l = c2pool.tile([128, GRP * 512], BF16, tag="c2l")
                    for h in range(GRP):
                        idx = g * GRP + h
                        q = order[idx]
                        hs = h * 512
                        t1 = t1pool.tile([128, QUAD * N], BF16, tag="t1")
                        if idx < N_GQ:
                            # GQ path: t1 = relu(z1) fused; 0.01*z1 linear
                            # part rides the GQ matmul, stationary 0.99*W2
                            for k in range(QUAD):
                                p = q * QUAD + k
                                nc.vector.tensor_scalar(
                                    t1[:, k * N:(k + 1) * N], vb,
                                    u2[:, b * NPAIR + p:b * NPAIR + p + 1], 0.0,
                                    op0=ALU.add, op1=ALU.max)
                            mqs = 512 * (NQ * b + q)
                            nc.tensor.matmul(Cps[:, hs:hs + 512], GQ,
                                             MQ[:, mqs:mqs + 512],
                                             start=True, stop=False)
                            nc.tensor.matmul(Cps[:, hs:hs + 512], W2a, t1[:, :],
                                             start=False, stop=True)
                        else:
                            # DVE path: exact lrelu on DVE, one matmul only
                            z1 = t1pool.tile([128, QUAD * N], BF16, tag="z1")
                            for k in range(QUAD):
                                p = q * QUAD + k
                                nc.vector.tensor_scalar(
                                    z1[:, k * N:(k + 1) * N], vb,
                                    u2[:, b * NPAIR + p:b * NPAIR + p + 1], None,
                                    op0=ALU.add)
                            nc.vector.scalar_tensor_tensor(
                                t1[:, :], z1[:, :], SLOPE, z1[:, :],
                                op0=ALU.mult, op1=ALU.max)
                            nc.tensor.matmul(Cps[:, hs:hs + 512], W2b, t1[:, :],
                                             start=True, stop=True)
                    bias = ccv if g * GRP < N_GQ else b2v
                    nc.scalar.activation(c2l[:, :], Cps[:, :], ACTF.Prelu,
                                         bias=bias, scale=1.0, alpha=SLOPE)
                    for h in range(GRP):
                        idx = g * GRP + h
                        pending.append((S, order[idx], idx,
                                        c2l[:, h * 512:(h + 1) * 512], b))
                    while len(pending) > L3LAG:
                        emit_l3(pending.pop(0))
            while pending:
                emit_l3(pending.pop(0))

        # ---------------- outputs ------------------------------------------
        nc.sync.dma_start(
            out[:, :, :].rearrange("b n c -> n b c")[:, :, 0:1], xb1)
        nc.sync.dma_start(
            out[:, :, :].rearrange("b n c -> n b c")[:, :, 1:2], val_cols[:, :])


# ---------------- host side -------------------------------------------------

def _lrelu(x):
    return np.where(x > 0, x, SLOPE * x)


def _bf16(a):
    import ml_dtypes
    return np.asarray(a, np.float32).astype(ml_dtypes.bfloat16)


def _pack_consts(x_core, Wn1, bn1, Wn2, bn2, Wno, bno,
                 Wc1, bc1, Wc2, bc2, Wco, bco, A_param):
    """Build (cf, cb, mq) for one core (x_core = [BL, N, D])."""
    cf = np.zeros((128, CF_W), np.float32)
    cbf = np.zeros((128, CB_W), np.float32)
    mqf = np.zeros((6, MQ_W), np.float32)

    Wc1a, Wc1b = Wc1[:D], Wc1[D:]          # [2, 64] each

    # u2: col 64*b + p -> [u_{2p} ; u_{2p+1}], u_i = Wc1a^T x_i + bc1
    u = x_core @ Wc1a + bc1                # [BL, N, 64]
    ue = u.reshape(BL, NPAIR, 2, H)
    u2 = np.concatenate([ue[:, :, 0, :], ue[:, :, 1, :]], axis=-1)  # [BL,64,128]
    cf[:, OFF_U2:OFF_U2 + BL * NPAIR] = u2.reshape(BL * NPAIR, 128).T

    # adjacency (fp64 sigmoid like the reference)
    z = A_param.astype(np.float64) - np.eye(N, dtype=np.float64) / EPS
    A = np.where(z >= 0, 1.0 / (1.0 + np.exp(-np.clip(z, 0, None))),
                 np.exp(np.clip(z, None, 0)) / (1.0 + np.exp(np.clip(z, None, 0))))
    A = A.astype(np.float32)

    # Mmask[i, 128k + j] = A[i, j] * (k == ((i % 8) >> 1))
    MM = np.zeros((N, 4, N), np.float32)
    ii = np.arange(N)
    MM[ii, (ii % 8) >> 1, :] = A
    cf[:, OFF_MM:OFF_MM + 512] = MM.reshape(N, 512)

    # node MLP on host + bco*rowsum(A)
    hn = _lrelu(x_core @ Wn1 + bn1)
    hn = _lrelu(hn @ Wn2 + bn2)
    node = (hn @ Wno)[..., 0] + bno[0]                   # [BL, N]
    cf[:, OFF_NODE:OFF_NODE + BL] = node.T + (bco[0] * A.sum(axis=1))[:, None]

    cf[:, OFF_XB1:OFF_XB1 + BL] = x_core[:, :, 1].T

    cc = SLOPE * (bc1 @ Wc2) + bc2                       # [64]
    cf[0:H, OFF_CC] = cc
    cf[H:2 * H, OFF_CC] = cc
    cf[0:H, OFF_B2] = bc2
    cf[H:2 * H, OFF_B2] = bc2

    # vv: col 128*b + j -> [v_j ; v_j], v_j = Wc1b^T x_j
    v = x_core @ Wc1b                                    # [BL, N, 64]
    vT = v.reshape(BN, H).T
    cbf[0:H, OFF_VV:OFF_VV + BN] = vT
    cbf[H:2 * H, OFF_VV:OFF_VV + BN] = vT

    cbf[0:H, OFF_W2A:OFF_W2A + H] = (1.0 - SLOPE) * Wc2
    cbf[H:2 * H, OFF_W2A + H:OFF_W2A + 2 * H] = (1.0 - SLOPE) * Wc2
    cbf[0:H, OFF_W2B:OFF_W2B + H] = Wc2
    cbf[H:2 * H, OFF_W2B + H:OFF_W2B + 2 * H] = Wc2

    Ga = Wc1a @ Wc2                                      # [2, 64]
    Gb = Wc1b @ Wc2
    cbf[0:2, OFF_GQ:OFF_GQ + H] = SLOPE * Gb
    cbf[0:2, OFF_GQ + H:OFF_GQ + 2 * H] = SLOPE * Gb
    cbf[2:4, OFF_GQ:OFF_GQ + H] = SLOPE * Ga
    cbf[4:6, OFF_GQ + H:OFF_GQ + 2 * H] = SLOPE * Ga

    # strip: cols 120..127 = alternating [Wco;0] / [0;Wco]
    for m in range(8):
        e = m & 1
        cbf[e * H:(e + 1) * H, OFF_STRIP + 8 * (NQ - 1) + m] = Wco[:, 0]

    # MQ moving tiles: col 512*(16b+q) + 128k + j
    #   rows 0:2 = x[b, j, :], rows 2:4 = x[b, 2p, :], rows 4:6 = x[b, 2p+1, :]
    xj = x_core[:, None, None, :, :]                       # [BL,1,1,N,2]
    xj = np.broadcast_to(xj, (BL, NQ, QUAD, N, 2))
    mqf[0:2] = xj.reshape(-1, 2).T
    xp = x_core.reshape(BL, NPAIR, 2, 2)                   # [BL,p,e,d]
    xi = xp.reshape(BL, NQ, QUAD, 1, 2, 2)
    xi = np.broadcast_to(xi, (BL, NQ, QUAD, N, 2, 2))
    mqf[2:4] = xi[..., 0, :].reshape(-1, 2).T
    mqf[4:6] = xi[..., 1, :].reshape(-1, 2).T

    return cf, _bf16(cbf), _bf16(mqf)


_CACHED_NC = None


def _get_nc():
    global _CACHED_NC
    if _CACHED_NC is None:
        _CACHED_NC = build_program()
    return _CACHED_NC


def make_in_maps(x, Wn1, bn1, Wn2, bn2, Wno, bno,
                 Wc1, bc1, Wc2, bc2, Wco, bco, A_param, t=None, **_unused):
    x = np.asarray(x, np.float32)
    args = [np.asarray(a, np.float32) for a in
            (Wn1, bn1, Wn2, bn2, Wno, bno, Wc1, bc1, Wc2, bc2, Wco, bco, A_param)]
    maps = []
    for c in range(NCORES):
        cf, cb, mq = _pack_consts(x[c * BL:(c + 1) * BL], *args)
        maps.append({"cf": cf, "cb": cb, "mq": mq})
    return maps


def kernel(**inputs):
    in_maps = make_in_maps(**inputs)
    nc = _get_nc()
    res = run_bass_kernel_spmd(nc, in_maps, list(range(NCORES)))
    out = np.concatenate([res.results[c]["out"] for c in range(NCORES)], axis=0)
    return out.astype(np.float32)


# revision 21
# speedup vs baseline: 1.1793x; 1.1793x over previous
"""Trainium2 Bass kernel for nn_NetworkODEModel (gnn_message_passing).

Reference computation (B=64, N=128, D=2, H=64):
  node_out = MLP_node(x)                                  # [B,N,1]
  c[b,i,j] = MLP_coup(cat(x[b,i], x[b,j]))                # [B,N,N,1]
  A        = sigmoid(A_param - I/eps)
  coup[b,i] = sum_j A[i,j] * c[b,i,j]
  out[...,0] = x[...,1];  out[...,1] = node_out + coup

Data-parallel over batch (8 cores x 8 batches); all O(B*N^2*H) work stays
in SBUF in bf16.  Per-quad tile = [128 part, 512 cols]: partitions carry two
i-streams (rows 0:64 = features of i=2p, 64:128 = i=2p+1), columns carry 4
pairs x 128 j.

Engine balance (the whole point of this structure).  Two interchangeable
layer-1 paths, split N_GQ / (16 - N_GQ) quads per batch to balance PE vs DVE:
  * GQ path: LeakyReLU(z) = 0.99*relu(z) + 0.01*z.  t1 = relu(v_j + u_i) =
    ONE dual-op tensor_scalar (op0=add, op1=max 0) per pair -- 4x DVE mode in
    bf16.  The 0.01*z linear part is rank-6 in (x_i, x_j) and rides a tiny
    accumulating matmul (GQ, 6-row stationary, moving tile MQ) into the same
    PSUM bank as layer 2 (stationary 0.99*blockdiag(W2,W2), ACT bias cc).
  * DVE path: exact lrelu on DVE -- 4x tensor_scalar z1-builds plus ONE
    scalar_tensor_tensor max(0.01*z1, z1) over the 512-wide quad, then a
    single unscaled blockdiag(W2,W2) matmul (ACT bias bc2).  One less PE
    matmul + LDWEIGHTS per quad at the cost of ~330ns more DVE.
  * Layer-2 LeakyReLU = ONE ScalarE Prelu(alpha=0.01, bias per path) per
    2-quad [128,1024] PSUM supertile, straight to SBUF bf16 -- ACT is the
    only cheap PSUM evictor (DVE fp32-PSUM ops run at 1x), and grouping
    amortizes its ~440-cycle access-latency init.
  * Layer 3 (Wco contraction + scatter to S[i,j]) = ONE [128,512] matmul per
    quad against a sliding 8-wide Wco strip, PSUM-accumulated over quads.
    Off-block columns produce garbage that the epilogue masks for free:
    coup = sum((S * Mmask), j) with Mmask[i,128k+j] = A[i,j]*(k == k(i)).
  * Per-batch epilogue on DVE: S*Mmask multiply, free-axis reduce, node add.
PE's L3 is software-pipelined (lags L3LAG quads) so PE rarely waits on ACT.
Host precomputes every linear map (u, v, GQ/W2a/W2b/strip/Mmask/node).
walrus encodes at most ONE sync wait per instruction -> _split_multiwaits
hoists extras onto same-engine NoOps.
"""

import sys

for _p in ("/opt/trn_rl_repo",):
    if _p not in sys.path:
        sys.path.insert(0, _p)

import numpy as np

import concourse.bass as bass
import concourse.mybir as mybir
import concourse.tile as tile
from concourse.bass_utils import run_bass_kernel_spmd

F32 = mybir.dt.float32
BF16 = mybir.dt.bfloat16
ALU = mybir.AluOpType
ACTF = mybir.ActivationFunctionType

NCORES = 8
B, N, D, H = 64, 128, 2, 64
BL = B // NCORES            # batches per core = 8
NPAIR = N // 2              # i-pairs per batch = 64
QUAD = 4                    # i-pairs per tile
NQ = NPAIR // QUAD          # 16 quads per batch
EPS = 1e-5
SLOPE = 0.01                # torch LeakyReLU default
L3LAG = 3                   # quads of software pipelining for the L3 matmul
GRP = 2                     # quads per ACT activation (PSUM supertile)
N_GQ = 8                    # quads per batch on the GQ-matmul path (rest DVE;
                            # must be a multiple of GRP so ACT groups stay
                            # bias-homogeneous).  8/8 balances PE vs DVE:
                            # measured 73us vs 138us (16/0) and 190us (0/16)
ALT_GROUPS = True           # alternate GQ/DVE ACT-groups through the batch
                            # (vs first-N_GQ block) for instantaneous balance

BN = BL * N                 # 1024 (b,j) columns per core
STRIPW = 8 * (NQ - 1) + 128  # 248: sliding 8-wide Wco window

# ---- f32 constants layout [128, CF_W] ----
OFF_U2 = 0                  # [128, 512]  u vectors, col = 64*b + p
OFF_MM = 512                # [128, 512]  A-mask for the S epilogue
OFF_NODE = 1024             # [128, 8]    node_out + bco*rowsum(A), [i, b]
OFF_XB1 = 1032              # [128, 8]    x[b, n, 1] as [n, b]
OFF_CC = 1040               # [128, 1]    layer-2 bias, GQ path (ACT bias port)
OFF_B2 = 1041               # [128, 1]    layer-2 bias, DVE-lrelu path
CF_W = 1042

# ---- bf16 constants layout [128, CB_W] ----
OFF_VV = 0                  # [128, 1024] [v_j; v_j], col = 128*b + j
OFF_W2A = 1024              # [128, 128]  0.99 * blockdiag(W2, W2)   (GQ path)
OFF_GQ = 1152               # [6, 128]    0.01 * [Gb; Ga|0; 0|Ga]
OFF_STRIP = 1280            # [128, 248]  sliding Wco strip
OFF_W2B = 1528              # [128, 128]  blockdiag(W2, W2)          (DVE path)
CB_W = 1656

MQ_W = BL * NQ * 512        # 65536 moving columns for the GQ matmul


def build_program(debug=False, split_waits=True, repeat=1):
    nc = bass.Bass("TRN2", target_bir_lowering=False, debug=debug)
    cf = nc.dram_tensor("cf", [128, CF_W], F32, kind="ExternalInput")
    cb = nc.dram_tensor("cb", [128, CB_W], BF16, kind="ExternalInput")
    mq = nc.dram_tensor("mq", [6, MQ_W], BF16, kind="ExternalInput")
    out = nc.dram_tensor("out", [BL, N, 2], F32, kind="ExternalOutput")

    with tile.TileContext(nc) as tc:
        _body(nc, tc, cf, cb, mq, out, repeat=repeat)
    if split_waits:
        _split_multiwaits(nc)
    nc.finalize()
    return nc


def _split_multiwaits(nc):
    """walrus on this stack encodes at most ONE sync wait per instruction;
    hoist extras onto same-engine NoOps."""
    import bass_rust
    n = 0
    for fn in nc.m.functions:
        for bb in fn.blocks:
            insts = bb.instructions
            changed = False
            out_list = []
            for inst in insts:
                si = inst.sync_info
                if si is not None and len(si.on_wait) > 1:
                    waits = list(si.on_wait)
                    for w in waits[:-1]:
                        nop = bass_rust.InstNoOp(name=f"ant-wait-split-{n}")
                        n += 1
                        nop.engine = inst.engine
                        nop.sync_info = bass_rust.SyncInfo(on_wait=[w], on_update=[])
                        out_list.append(nop)
                    inst.sync_info = bass_rust.SyncInfo(
                        on_wait=[waits[-1]], on_update=list(si.on_update))
                    changed = True
                out_list.append(inst)
            if changed:
                bb.instructions = out_list


def _body(nc, tc, cf, cb, mq, out, repeat=1):
    with (
        tc.tile_pool(name="const", bufs=1) as cpool,
        tc.tile_pool(name="t1p", bufs=6) as t1pool,
        tc.tile_pool(name="c2p", bufs=3) as c2pool,
        tc.tile_pool(name="zp", bufs=2) as zpool,
        tc.tile_pool(name="psum_c", bufs=3, space="PSUM") as ppool,
        tc.tile_pool(name="psum_s", bufs=2, space="PSUM") as spool,
    ):
        CF = cpool.tile([128, CF_W], F32, tag="cf")
        CB = cpool.tile([128, CB_W], BF16, tag="cb")
        MQ = cpool.tile([6, MQ_W], BF16, tag="mq")
        nc.sync.dma_start(CF[:, :], cf[:, :])
        nc.sync.dma_start(CB[:, :], cb[:, :])
        nc.sync.dma_start(MQ[:, :], mq[:, :])
        # absorb each DMA wait on DVE once so later DVE readers never pair a
        # DMA wait with a second wait
        dscr = cpool.tile([128, 2], F32, tag="dscr")
        nc.vector.tensor_copy(dscr[:, 0:1], CF[:, 0:1])
        nc.vector.tensor_copy(dscr[:, 1:2], CB[:, 0:1])
        nc.vector.tensor_copy(dscr[0:6, 0:1], MQ[:, 0:1])

        u2 = CF[:, OFF_U2:OFF_U2 + BL * NPAIR]
        Mmask = CF[:, OFF_MM:OFF_MM + 512]
        nodec = CF[:, OFF_NODE:OFF_NODE + BL]
        xb1 = CF[:, OFF_XB1:OFF_XB1 + BL]
        ccv = CF[:, OFF_CC:OFF_CC + 1]
        b2v = CF[:, OFF_B2:OFF_B2 + 1]
        vv = CB[:, OFF_VV:OFF_VV + BN]
        W2a = CB[:, OFF_W2A:OFF_W2A + 128]
        GQ = CB[0:6, OFF_GQ:OFF_GQ + 128]
        strip = CB[:, OFF_STRIP:OFF_STRIP + STRIPW]
        W2b = CB[:, OFF_W2B:OFF_W2B + 128]

        val_cols = cpool.tile([N, BL], F32, tag="val_cols")

        for _rep in range(repeat):
            pending = []   # (S_tile, q, c2l, b) awaiting L3 emission

            def emit_l3(job):
                S, q, idx, c2l, b = job
                nc.tensor.matmul(
                    S[:, :], strip[:, 8 * (NQ - 1 - q):8 * (NQ - 1 - q) + 128],
                    c2l[:, :], start=(idx == 0), stop=(idx == NQ - 1))
                if idx == NQ - 1:
                    # epilogue: coup = sum_j A*S (+ node column)
                    Z = zpool.tile([128, 512], F32, tag="Z")
                    nc.vector.tensor_tensor(Z[:, :], S[:, :], Mmask, op=ALU.mult)
                    rs = zpool.tile([128, 1], F32, tag="rs")
                    nc.vector.tensor_reduce(rs[:, :], Z[:, :],
                                            axis=mybir.AxisListType.X, op=ALU.add)
                    nc.vector.tensor_scalar(val_cols[:, b:b + 1], rs[:, :],
                                            nodec[:, b:b + 1], None, op0=ALU.add)

            # N_GQ quads take the GQ-matmul path, the rest the DVE path;
            # each 2-quad ACT group stays path-homogeneous (the two paths
            # need different layer-2 bias vectors)
            ngrp = NQ // GRP
            ngq_grp = N_GQ // GRP
            if ALT_GROUPS and 0 < ngq_grp < ngrp:
                gq_groups = set(
                    round(i * ngrp / ngq_grp) for i in range(ngq_grp))
            else:
                gq_groups = set(range(ngq_grp))
            order = list(range(NQ))
            for b in range(BL):
                S = spool.tile([128, 512], F32, tag="S")
                vb = vv[:, b * N:(b + 1) * N]
                for g in range(NQ // GRP):
                    # 2-quad supertile: matmuls fill both 512-col halves of a
                    # 2-bank PSUM tile; ONE ACT Prelu drains all 1024 cols
                    Cps = ppool.tile([128, GRP * 512], F32, tag="Cps")
                    c2l = c2pool.tile([128, GRP * 512], BF16, tag="c2l")
                    for h in range(GRP):
                        idx = g * GRP + h
                        q = order[idx]
                        hs = h * 512
                        t1 = t1pool.tile([128, QUAD * N], BF16, tag="t1")
                        if g in gq_groups:
                            # GQ path: t1 = relu(z1) fused; 0.01*z1 linear
                            # part rides the GQ matmul, stationary 0.99*W2
                            for k in range(QUAD):
                                p = q * QUAD + k
                                nc.vector.tensor_scalar(
                                    t1[:, k * N:(k + 1) * N], vb,
                                    u2[:, b * NPAIR + p:b * NPAIR + p + 1], 0.0,
                                    op0=ALU.add, op1=ALU.max)
                            mqs = 512 * (NQ * b + q)
                            nc.tensor.matmul(Cps[:, hs:hs + 512], GQ,
                                             MQ[:, mqs:mqs + 512],
                                             start=True, stop=False)
                            nc.tensor.matmul(Cps[:, hs:hs + 512], W2a, t1[:, :],
                                             start=False, stop=True)
                        else:
                            # DVE path: exact lrelu on DVE, one matmul only
                            z1 = t1pool.tile([128, QUAD * N], BF16, tag="z1")
                            for k in range(QUAD):
                                p = q * QUAD + k
                                nc.vector.tensor_scalar(
                                    z1[:, k * N:(k + 1) * N], vb,
                                    u2[:, b * NPAIR + p:b * NPAIR + p + 1], None,
                                    op0=ALU.add)
                            nc.vector.scalar_tensor_tensor(
                                t1[:, :], z1[:, :], SLOPE, z1[:, :],
                                op0=ALU.mult, op1=ALU.max)
                            nc.tensor.matmul(Cps[:, hs:hs + 512], W2b, t1[:, :],
                                             start=True, stop=True)
                    bias = ccv if g in gq_groups else b2v
                    nc.scalar.activation(c2l[:, :], Cps[:, :], ACTF.Prelu,
                                         bias=bias, scale=1.0, alpha=SLOPE)
                    for h in range(GRP):
                        idx = g * GRP + h
                        pending.append((S, order[idx], idx,
                                        c2l[:, h * 512:(h + 1) * 512], b))
                    while len(pending) > L3LAG:
                        emit_l3(pending.pop(0))
            while pending:
                emit_l3(pending.pop(0))

        # ---------------- outputs ------------------------------------------
        nc.sync.dma_start(
            out[:, :, :].rearrange("b n c -> n b c")[:, :, 0:1], xb1)
        nc.sync.dma_start(
            out[:, :, :].rearrange("b n c -> n b c")[:, :, 1:2], val_cols[:, :])


# ---------------- host side -------------------------------------------------

def _lrelu(x):
    return np.where(x > 0, x, SLOPE * x)


def _bf16(a):
    import ml_dtypes
    return np.asarray(a, np.float32).astype(ml_dtypes.bfloat16)


def _pack_consts(x_core, Wn1, bn1, Wn2, bn2, Wno, bno,
                 Wc1, bc1, Wc2, bc2, Wco, bco, A_param):
    """Build (cf, cb, mq) for one core (x_core = [BL, N, D])."""
    cf = np.zeros((128, CF_W), np.float32)
    cbf = np.zeros((128, CB_W), np.float32)
    mqf = np.zeros((6, MQ_W), np.float32)

    Wc1a, Wc1b = Wc1[:D], Wc1[D:]          # [2, 64] each

    # u2: col 64*b + p -> [u_{2p} ; u_{2p+1}], u_i = Wc1a^T x_i + bc1
    u = x_core @ Wc1a + bc1                # [BL, N, 64]
    ue = u.reshape(BL, NPAIR, 2, H)
    u2 = np.concatenate([ue[:, :, 0, :], ue[:, :, 1, :]], axis=-1)  # [BL,64,128]
    cf[:, OFF_U2:OFF_U2 + BL * NPAIR] = u2.reshape(BL * NPAIR, 128).T

    # adjacency (fp64 sigmoid like the reference)
    z = A_param.astype(np.float64) - np.eye(N, dtype=np.float64) / EPS
    A = np.where(z >= 0, 1.0 / (1.0 + np.exp(-np.clip(z, 0, None))),
                 np.exp(np.clip(z, None, 0)) / (1.0 + np.exp(np.clip(z, None, 0))))
    A = A.astype(np.float32)

    # Mmask[i, 128k + j] = A[i, j] * (k == ((i % 8) >> 1))
    MM = np.zeros((N, 4, N), np.float32)
    ii = np.arange(N)
    MM[ii, (ii % 8) >> 1, :] = A
    cf[:, OFF_MM:OFF_MM + 512] = MM.reshape(N, 512)

    # node MLP on host + bco*rowsum(A)
    hn = _lrelu(x_core @ Wn1 + bn1)
    hn = _lrelu(hn @ Wn2 + bn2)
    node = (hn @ Wno)[..., 0] + bno[0]                   # [BL, N]
    cf[:, OFF_NODE:OFF_NODE + BL] = node.T + (bco[0] * A.sum(axis=1))[:, None]

    cf[:, OFF_XB1:OFF_XB1 + BL] = x_core[:, :, 1].T

    cc = SLOPE * (bc1 @ Wc2) + bc2                       # [64]
    cf[0:H, OFF_CC] = cc
    cf[H:2 * H, OFF_CC] = cc
    cf[0:H, OFF_B2] = bc2
    cf[H:2 * H, OFF_B2] = bc2

    # vv: col 128*b + j -> [v_j ; v_j], v_j = Wc1b^T x_j
    v = x_core @ Wc1b                                    # [BL, N, 64]
    vT = v.reshape(BN, H).T
    cbf[0:H, OFF_VV:OFF_VV + BN] = vT
    cbf[H:2 * H, OFF_VV:OFF_VV + BN] = vT

    cbf[0:H, OFF_W2A:OFF_W2A + H] = (1.0 - SLOPE) * Wc2
    cbf[H:2 * H, OFF_W2A + H:OFF_W2A + 2 * H] = (1.0 - SLOPE) * Wc2
    cbf[0:H, OFF_W2B:OFF_W2B + H] = Wc2
    cbf[H:2 * H, OFF_W2B + H:OFF_W2B + 2 * H] = Wc2

    Ga = Wc1a @ Wc2                                      # [2, 64]
    Gb = Wc1b @ Wc2
    cbf[0:2, OFF_GQ:OFF_GQ + H] = SLOPE * Gb
    cbf[0:2, OFF_GQ + H:OFF_GQ + 2 * H] = SLOPE * Gb
    cbf[2:4, OFF_GQ:OFF_GQ + H] = SLOPE * Ga
    cbf[4:6, OFF_GQ + H:OFF_GQ + 2 * H] = SLOPE * Ga

    # strip: cols 120..127 = alternating [Wco;0] / [0;Wco]
    for m in range(8):
        e = m & 1
        cbf[e * H:(e + 1) * H, OFF_STRIP + 8 * (NQ - 1) + m] = Wco[:, 0]

    # MQ moving tiles: col 512*(16b+q) + 128k + j
    #   rows 0:2 = x[b, j, :], rows 2:4 = x[b, 2p, :], rows 4:6 = x[b, 2p+1, :]
    xj = x_core[:, None, None, :, :]                       # [BL,1,1,N,2]
    xj = np.broadcast_to(xj, (BL, NQ, QUAD, N, 2))
    mqf[0:2] = xj.reshape(-1, 2).T
    xp = x_core.reshape(BL, NPAIR, 2, 2)                   # [BL,p,e,d]
    xi = xp.reshape(BL, NQ, QUAD, 1, 2, 2)
    xi = np.broadcast_to(xi, (BL, NQ, QUAD, N, 2, 2))
    mqf[2:4] = xi[..., 0, :].reshape(-1, 2).T
    mqf[4:6] = xi[..., 1, :].reshape(-1, 2).T

    return cf, _bf16(cbf), _bf16(mqf)


_CACHED_NC = None


def _get_nc():
    global _CACHED_NC
    if _CACHED_NC is None:
        _CACHED_NC = build_program()
    return _CACHED_NC


def make_in_maps(x, Wn1, bn1, Wn2, bn2, Wno, bno,
                 Wc1, bc1, Wc2, bc2, Wco, bco, A_param, t=None, **_unused):
    x = np.asarray(x, np.float32)
    args = [np.asarray(a, np.float32) for a in
            (Wn1, bn1, Wn2, bn2, Wno, bno, Wc1, bc1, Wc2, bc2, Wco, bco, A_param)]
    maps = []
    for c in range(NCORES):
        cf, cb, mq = _pack_consts(x[c * BL:(c + 1) * BL], *args)
        maps.append({"cf": cf, "cb": cb, "mq": mq})
    return maps


def kernel(**inputs):
    in_maps = make_in_maps(**inputs)
    nc = _get_nc()
    res = run_bass_kernel_spmd(nc, in_maps, list(range(NCORES)))
    out = np.concatenate([res.results[c]["out"] for c in range(NCORES)], axis=0)
    return out.astype(np.float32)
